# revision 14
# baseline (speedup 1.0000x reference)
"""Trainium2 Bass kernel for gated multi-head attention (nn_MHAtt_41274635714591).

Strategy: data-parallel over batch — 8 batches onto 8 NeuronCores, one batch per
core, no collectives. Per core (S=1024, D=1024, H=8, DB=128):

Per-head software pipeline keeps PE busy while ACT (softmax exp + gate
activations) runs one head behind:

  prologue: xTq/xTk transposed loads (PE transposes), head-0 q/k projections,
            head-0 gates; head-0 scores interleaved with xTv transposes; v
            projection for heads 0-3.
  iteration h (1..7): prefetch W blocks for h+1; project q/k head h; gate
            head h; then 8 units of [scores(h, kj) + pv(h-1, qi)] so the
            psum ring never waits on the exp chain; v projection for heads
            4-7 trickles through iterations 1-4 as extra PE filler.
  epilogue: pv(7), merge with streamed Wm + bm, DMA out.

Device-side details:
  - All inputs arrive bf16 (host-converted); biases / mask arrive as
    host-prepped f32 per-partition tensors. No device casts, no SWDGE.
  - qhT/khT are built transposed ([d_head, s]) via lhsT=W-col-block,
    rhs=xT; vh is natural [s, d] with an extra all-ones column so the PV
    matmul yields the softmax denominator for free.
  - Gate MLP sigmoid is computed as 0.5 + 0.5*tanh(z/2): tanh lives in the
    same ACT table set as exp and identity, so the whole kernel runs on one
    table set (zero ~2.7us table switches).
  - Scores are computed transposed S^T[k,q]; exp(scale*x + maskbias_k) on
    ACT writes P^T directly; mask folds in as per-partition additive bias.
  - PV: out[q, 0:129] = sum_k P^T-chunk^T @ vh_aug; col 128 = denominator;
    two q-tiles share one PSUM bank and one reciprocal+normalize DVE pass.

The harness calls kernel(**full_inputs); we shard batch across cores with
run_bass_kernel_spmd and stack the per-core outputs.
"""

import math
import os
import sys

for _p in ("/opt/trn_rl_repo", "/root/.axon_site/_ro/trn_rl_repo"):
    if os.path.isdir(_p) and _p not in sys.path:
        sys.path.insert(0, _p)

import numpy as np
import ml_dtypes

import concourse.bass as bass
import concourse.mybir as mybir
import concourse.tile as tile
from concourse import bacc
from concourse.masks import make_identity

F32 = mybir.dt.float32
BF16 = mybir.dt.bfloat16
F8 = mybir.dt.float8e4
PM_DR = mybir.MatmulPerfMode.DoubleRow
AF = mybir.ActivationFunctionType
OP = mybir.AluOpType

B, S, D, H = 8, 1024, 1024, 8
DB = D // H          # 128 per-head dim
P = 128              # partitions
KJ = S // P          # 8 tiles of 128 along s
NDT = D // P         # 8 tiles of 128 along d
SCALE = 1.0 / math.sqrt(DB)
NEG = -1e9

NP_BF16 = ml_dtypes.bfloat16
NP_F8 = ml_dtypes.float8_e4m3
W8_SCALE = 64.0  # host premultiplier lifting 0.02-std weights out of fp8-e4m3
                 # subnormal range; undone by 1/64 at psum eviction


def build_nc(repeat=1):
    pdt = BF16
    adt = BF16
    # Bacc (not plain Bass): its compile pipeline fuses multi-sem waits into
    # event semaphores — this container's walrus rejects instructions carrying
    # more than one sync wait — and inserts GPSIMD library / ACT table loads.
    nc = bacc.Bacc()

    qT_d = nc.dram_tensor("qT", [P, NDT, S], F8, kind="ExternalInput")
    kT_d = nc.dram_tensor("kT", [P, NDT, S], F8, kind="ExternalInput")
    vT_d = nc.dram_tensor("vT", [P, NDT, S], BF16, kind="ExternalInput")
    maskb_d = nc.dram_tensor("maskb", [P, KJ], F32, kind="ExternalInput")
    Wq = nc.dram_tensor("Wq", [D, D], F8, kind="ExternalInput")
    Wk = nc.dram_tensor("Wk", [D, D], F8, kind="ExternalInput")
    Wv = nc.dram_tensor("Wv", [D, D], BF16, kind="ExternalInput")
    Wm = nc.dram_tensor("Wm", [D, D], BF16, kind="ExternalInput")
    bq_d = nc.dram_tensor("bq_sb", [P, NDT], F32, kind="ExternalInput")
    bk_d = nc.dram_tensor("bk_sb", [P, NDT], F32, kind="ExternalInput")
    bv_d = nc.dram_tensor("bv_rep", [P, D], F32, kind="ExternalInput")
    bm_d = nc.dram_tensor("bm_rep", [P, D], F32, kind="ExternalInput")
    WgX_d = nc.dram_tensor("WgX_sb", [DB, DB], BF16, kind="ExternalInput")
    WgY_d = nc.dram_tensor("WgY_sb", [DB, DB], BF16, kind="ExternalInput")
    Wg2c_d = nc.dram_tensor("Wg2c", [P, 2, P], BF16, kind="ExternalInput")
    bgX_d = nc.dram_tensor("bgX_sb", [P, 1], F32, kind="ExternalInput")
    bgY_d = nc.dram_tensor("bgY_sb", [P, 1], F32, kind="ExternalInput")
    bg2h_d = nc.dram_tensor("bg2h", [P, 2], F32, kind="ExternalInput")
    out = nc.dram_tensor("out", [S, D], F32, kind="ExternalOutput")

    from contextlib import ExitStack

    with tile.TileContext(nc) as tc, ExitStack() as ctx:
        consts = ctx.enter_context(tc.tile_pool(name="consts", bufs=1))
        persist = ctx.enter_context(tc.tile_pool(name="persist", bufs=1))
        xslab = ctx.enter_context(tc.tile_pool(name="xslab", bufs=3))
        ptslab = ctx.enter_context(tc.tile_pool(name="ptslab", bufs=2))
        xrow = ctx.enter_context(tc.tile_pool(name="xrow", bufs=3))
        wqk = ctx.enter_context(tc.tile_pool(name="wqk", bufs=4))
        wbig = ctx.enter_context(tc.tile_pool(name="wbig", bufs=2))
        gpool = ctx.enter_context(tc.tile_pool(name="gpool", bufs=1))
        attp = ctx.enter_context(tc.tile_pool(name="attp", bufs=2))
        smalls = ctx.enter_context(tc.tile_pool(name="smalls", bufs=2))
        outp = ctx.enter_context(tc.tile_pool(name="outp", bufs=2))
        brep = ctx.enter_context(tc.tile_pool(name="brep", bufs=1))
        # PSUM: psc 2x[128,1024]f32 (4 banks) + ppv 2x[128,2,129]f32 (2 banks)
        # + ptr 2x[128,1024]bf16 (2 banks) = 8 banks
        psc = ctx.enter_context(tc.tile_pool(name="psc", bufs=2, space="PSUM"))
        ppv = ctx.enter_context(tc.tile_pool(name="ppv", bufs=2, space="PSUM"))
        ptr = ctx.enter_context(tc.tile_pool(name="ptr", bufs=2, space="PSUM"))
        if repeat > 1:
            ctx.enter_context(tc.For_i(0, repeat, 1))

        # ---- constants (all host-prepped, plain HWDGE loads) ----
        identp = consts.tile([P, P], pdt, tag="identp")
        make_identity(nc, identp)

        maskb = consts.tile([P, KJ], F32, tag="maskb")
        nc.scalar.dma_start(out=maskb, in_=maskb_d[:, :])
        bq_sb = consts.tile([P, NDT], F32, tag="bq_sb")
        nc.scalar.dma_start(out=bq_sb, in_=bq_d[:, :])
        bk_sb = consts.tile([P, NDT], F32, tag="bk_sb")
        nc.scalar.dma_start(out=bk_sb, in_=bk_d[:, :])
        bgX_sb = consts.tile([P, 1], F32, tag="bgX_sb")
        nc.scalar.dma_start(out=bgX_sb, in_=bgX_d[:, :])
        bgY_sb = consts.tile([P, 1], F32, tag="bgY_sb")
        nc.scalar.dma_start(out=bgY_sb, in_=bgY_d[:, :])
        bg2h = consts.tile([P, 2], F32, tag="bg2h")
        nc.scalar.dma_start(out=bg2h, in_=bg2h_d[:, :])
        WgX_sb = consts.tile([P, DB], adt, tag="WgX_sb")
        nc.scalar.dma_start(out=WgX_sb, in_=WgX_d[:, :])
        WgY_sb = consts.tile([P, DB], adt, tag="WgY_sb")
        nc.scalar.dma_start(out=WgY_sb, in_=WgY_d[:, :])
        Wg2c = consts.tile([P, 2, P], adt, tag="Wg2c")
        nc.scalar.dma_start(out=Wg2c, in_=Wg2c_d[:, :, :])
        bv_rep = brep.tile([P, D], F32, tag="brep")
        nc.scalar.dma_start(out=bv_rep, in_=bv_d[:, :])

        # warm the ACT table set (exp_and_others) during startup DMA time
        warm = smalls.tile([P, 1], F32, tag="warm")
        nc.scalar.activation(warm, maskb[:, 0:1], AF.Identity)

        # ---- persistent activations ----
        qhT = persist.tile([P, H, S], adt, tag="qhT")   # [db, h, s] = (q@Wq+b)^T
        khT = persist.tile([P, H, S], adt, tag="khT")
        vh_aug = persist.tile([P, H, KJ, DB + 1], adt, tag="vh_aug")
        nc.vector.memset(vh_aug[:, :, :, DB : DB + 1], 1.0)
        A_T = persist.tile([P, H, S], pdt, tag="A_T")   # attention out, transposed

        # ---- helpers ----
        def load_xT(xTdram, dt=pdt, eng=None):
            # host-pretransposed x^T slab [d-in-tile, i, s]; chunked DMA so
            # the first projection matmuls start before the tail arrives
            xT = xslab.tile([P, NDT, S], dt, tag="xslab")
            for c in range(0, NDT, 2):
                (eng or nc.sync).dma_start(
                    out=xT[:, c : c + 2, :], in_=xTdram[:, c : c + 2, :]
                )
            return xT

        def load_w_head(Wdram, h):
            # one 128-col block of W: [d_in-tile, i, d_out 128] (fp8)
            wb = wqk.tile([P, NDT, DB], F8, tag="wqk")
            nc.sync.dma_start(
                out=wb,
                in_=Wdram[:, h * DB : (h + 1) * DB].rearrange(
                    "(i p) n -> p i n", p=P
                ),
            )
            return wb

        def load_w_half(Wdram, half, eng=None):
            # [D, 512] column-half of W, chunked so first use starts early
            wb = wbig.tile([P, NDT, 512], pdt, tag="wbig")
            wsrc = Wdram[:, half * 512 : (half + 1) * 512].rearrange(
                "(i p) n -> p i n", p=P
            )
            for c in range(0, NDT, 2):
                (eng or nc.sync).dma_start(
                    out=wb[:, c : c + 2, :], in_=wsrc[:, c : c + 2, :]
                )
            return wb

        def proj_head(xT, wb, bias_sb, h, dstT):
            # dstT[:, h, :] = (x @ W[:, hDB:(h+1)DB] + b_h)^T
            # fp8 DoubleRow: each matmul contracts a pair of 128-row blocks
            # (lhsT [P,2,DB], rhs [P,2,512] -> out [DB,512]) at ~1.4x bf16.
            for sh in range(2):
                sl = slice(sh * 512, (sh + 1) * 512)
                ps = psc.tile([P, 512], F32, tag="pacc")
                for i in range(0, NDT, 2):
                    nc.tensor.matmul(
                        ps,
                        wb[:, i : i + 2, :],
                        xT[:, i : i + 2, sl],
                        start=(i == 0),
                        stop=(i == NDT - 2),
                        perf_mode=PM_DR,
                    )
                nc.vector.tensor_scalar(
                    dstT[:, h, sl], ps, 1.0 / W8_SCALE, bias_sb[:, h : h + 1],
                    OP.mult, OP.add,
                )

        def vgroup_chunk(xTv, wch, g, m):
            # v projection for heads 4g..4g+3, s-tile m (natural layout)
            ps = psc.tile([P, 512], F32, tag="pacc")
            for i in range(NDT):
                nc.tensor.matmul(
                    ps,
                    xTv[:, i, m * P : (m + 1) * P],
                    wch[:, i, :],
                    start=(i == 0),
                    stop=(i == NDT - 1),
                )
            nc.vector.tensor_tensor(
                vh_aug[:, 4 * g : 4 * g + 4, m, 0:DB],
                ps.rearrange("p (h n) -> p h n", n=DB),
                bv_rep[:, g * 512 : (g + 1) * 512].rearrange(
                    "p (h n) -> p h n", n=DB
                ),
                OP.add,
            )

        def gates_xy(h):
            # first half of the gate MLP: gx = kh@WgX+bgX, gy = qh@WgY+bgY,
            # tt = gx*gy. Emitted early so the ACT chain drains while the PE
            # grinds score/pv units.
            gx = gpool.tile([P, S], adt, tag="gx")
            psx = psc.tile([P, S], F32, tag="pacc")
            for sh in range(2):
                sl = slice(sh * 512, (sh + 1) * 512)
                nc.tensor.matmul(
                    psx[:, sl], WgX_sb, khT[:, h, sl], start=True, stop=True
                )
            nc.vector.tensor_scalar_add(gx, psx, bgX_sb)
            gy = gpool.tile([P, S], adt, tag="gy")
            psy = psc.tile([P, S], F32, tag="pacc")
            for sh in range(2):
                sl = slice(sh * 512, (sh + 1) * 512)
                nc.tensor.matmul(
                    psy[:, sl], WgY_sb, qhT[:, h, sl], start=True, stop=True
                )
            nc.vector.tensor_scalar_add(gy, psy, bgY_sb)
            tt = gpool.tile([P, S], adt, tag="tt")
            nc.vector.tensor_tensor(tt, gx, gy, OP.mult)
            return tt

        def gates_z(h, tt):
            # second half: gate = sigmoid(tt@Wg2 + bg2) via
            # sigmoid(z) = 0.5 + 0.5*tanh(z/2) — stays in the exp table set.
            # z matmuls use replicated Wg2 columns: every output partition
            # carries the same gate row -> no cross-partition broadcast needed.
            for gi, dstT in ((0, khT), (1, qhT)):
                psz = psc.tile([P, S], F32, tag="pacc")
                for sh in range(2):
                    sl = slice(sh * 512, (sh + 1) * 512)
                    nc.tensor.matmul(
                        psz[:, sl], Wg2c[:, gi, :], tt[:, sl], start=True, stop=True
                    )
                t = gpool.tile([P, S], adt, tag=f"t{gi}")
                nc.scalar.activation(
                    t, psz, AF.Tanh, bias=bg2h[:, gi : gi + 1], scale=0.5
                )
                g = gpool.tile([P, S], adt, tag=f"g{gi}")
                nc.vector.tensor_scalar(g, t, 0.5, 0.5, OP.mult, OP.add)
                nc.vector.tensor_tensor(dstT[:, h, :], dstT[:, h, :], g, OP.mult)

        def score_unit(h, PT, kj):
            ps = psc.tile([P, S], F32, tag="pacc")
            for sh in range(2):
                sl = slice(sh * 512, (sh + 1) * 512)
                nc.tensor.matmul(
                    ps[:, sl],
                    khT[:, h, kj * P : (kj + 1) * P],
                    qhT[:, h, sl],
                    start=True,
                    stop=True,
                )
            nc.scalar.activation(
                PT[:, kj, :], ps, AF.Exp,
                bias=maskb[:, kj : kj + 1], scale=SCALE,
            )

        def pv_unit(h, PT, qi, pt2):
            # one q-tile of PV: 8 accumulating MMs, then normalize + transpose
            pv = ppv.tile([P, DB + 1], F32, tag="pv", name="pv")
            for kj in range(KJ):
                nc.tensor.matmul(
                    pv,
                    PT[:, kj, qi * P : (qi + 1) * P],
                    vh_aug[:, h, kj, :],
                    start=(kj == 0),
                    stop=(kj == KJ - 1),
                )
            rec = smalls.tile([P, 1], F32, tag="rec")
            nc.vector.reciprocal(rec, pv[:, DB : DB + 1])
            asb = attp.tile([P, P], pdt, tag="asb")
            nc.vector.tensor_scalar_mul(asb, pv[:, 0:DB], rec)
            nc.tensor.transpose(
                pt2[:, qi * P : (qi + 1) * P], asb, identp
            )

        # ================= emission schedule =================
        # Head pipeline: iteration h runs scores(h) + pv(h-1) on the PE in 8
        # units that each outlast one exp (so the psum ring never stalls on
        # ACT), while the PE-side prep for head h+1 (q/k projections, gate
        # matmuls) rides inside the units as extra filler. Gate ACT chains
        # for h+1 drain during iteration h — scores(h+1) never waits on them.
        # big loads fan out across engine DMA queues: x^T slabs on sync +
        # vector, v/m weight halves on scalar — a single queue can't reach
        # the core's aggregate HBM bandwidth
        wq = {0: load_w_head(Wq, 0)}
        wk = {0: load_w_head(Wk, 0)}
        wq[1] = load_w_head(Wq, 1)
        wk[1] = load_w_head(Wk, 1)
        xTq = load_xT(qT_d, dt=F8)
        xTk = load_xT(kT_d, dt=F8)
        xTv = load_xT(vT_d, eng=nc.scalar)
        wv0 = load_w_half(Wv, 0, eng=nc.scalar)
        proj_head(xTq, wq.pop(0), bq_sb, 0, qhT)
        proj_head(xTk, wk.pop(0), bk_sb, 0, khT)
        tt0 = gates_xy(0)
        proj_head(xTq, wq.pop(1), bq_sb, 1, qhT)
        proj_head(xTk, wk.pop(1), bk_sb, 1, khT)
        tt1 = gates_xy(1)
        gates_z(0, tt0)
        gates_z(1, tt1)
        vgroup_chunk(xTv, wv0, 0, 0)
        vgroup_chunk(xTv, wv0, 0, 1)

        # head-0 scores with v projection (heads 0-3) as filler
        PTs = {0: ptslab.tile([P, KJ, S], adt, tag="PT", name="PT0")}
        for j in range(KJ):
            score_unit(0, PTs[0], j)
            if j < 6:
                vgroup_chunk(xTv, wv0, 0, j + 2)

        # steady-state iterations: scores(h) + pv(h-1) + prep(h+1)
        wv1 = None
        wm = {}
        bm_rep = None
        for h in range(1, H):
            if h + 1 < H:
                wq[h + 1] = load_w_head(Wq, h + 1)
                wk[h + 1] = load_w_head(Wk, h + 1)
            if h == 1:
                wv1 = load_w_half(Wv, 1, eng=nc.scalar)
            if h == 5:
                bm_rep = brep.tile([P, D], F32, tag="brep")
                nc.scalar.dma_start(out=bm_rep, in_=bm_d[:, :])
            if h == 6:
                wm[0] = load_w_half(Wm, 0, eng=nc.scalar)
            if h == 7:
                wm[1] = load_w_half(Wm, 1, eng=nc.scalar)
            PTs[h] = ptslab.tile([P, KJ, S], adt, tag="PT", name=f"PT{h}")
            pt2 = ptr.tile([P, NDT * P], pdt, tag="trps")
            for j in range(KJ):
                score_unit(h, PTs[h], j)
                pv_unit(h - 1, PTs[h - 1], j, pt2)
                if h + 1 < H:
                    if j == 0:
                        proj_head(xTq, wq.pop(h + 1), bq_sb, h + 1, qhT)
                    elif j == 1:
                        proj_head(xTk, wk.pop(h + 1), bk_sb, h + 1, khT)
                    elif j == 2:
                        tt_next = gates_xy(h + 1)
                    elif j == 5:
                        gates_z(h + 1, tt_next)
                if 1 <= h <= 4 and j in (6, 7):
                    vgroup_chunk(xTv, wv1, 1, 2 * (h - 1) + (j - 6))
            nc.vector.tensor_copy(A_T[:, h - 1, :], pt2)
            PTs.pop(h - 1)

        # last head's PV
        pt2 = ptr.tile([P, NDT * P], pdt, tag="trps")
        for j in range(KJ):
            pv_unit(H - 1, PTs[H - 1], j, pt2)
        nc.vector.tensor_copy(A_T[:, H - 1, :], pt2)

        # ---- merge: out = A @ Wm + bm ----
        for m in range(KJ):
            ps = psc.tile([P, S], F32, tag="pacc")
            for half in range(2):
                sl = slice(half * 512, (half + 1) * 512)
                for i in range(NDT):
                    nc.tensor.matmul(
                        ps[:, sl],
                        A_T[:, i, m * P : (m + 1) * P],
                        wm[half][:, i, :],
                        start=(i == 0),
                        stop=(i == NDT - 1),
                    )
            osb = outp.tile([P, S], F32, tag="osb")
            nc.vector.tensor_tensor(osb, ps, bm_rep, OP.add)
            nc.sync.dma_start(out=out[m * P : (m + 1) * P, :], in_=osb)

    nc.finalize()
    return nc


_NC_CACHE = {}


def _get_nc(key="v2b"):
    if key not in _NC_CACHE:
        _NC_CACHE[key] = build_nc()
    return _NC_CACHE[key]


def _f32(a):
    return np.ascontiguousarray(np.asarray(a, dtype=np.float32))


def _bf16(a):
    return np.ascontiguousarray(np.asarray(a, dtype=np.float32).astype(NP_BF16))


def make_in_maps(v, k, q, mask, Wv, bv, Wk, bk, Wq, bq, Wm, bm,
                 WgX, bgX, WgY, bgY, Wg2, bg2):
    """Host-side prep: bf16 casts, bias rearranges, gate-weight replication.
    Returns one input map per core (batch b -> core b)."""
    nb = int(np.asarray(q).shape[0])
    Wg2_f = _f32(Wg2)
    def _w8(Wt):
        return np.ascontiguousarray(
            (np.asarray(Wt, dtype=np.float32) * W8_SCALE).astype(NP_F8)
        )

    shared = {
        "Wq": _w8(Wq), "Wk": _w8(Wk), "Wv": _bf16(Wv), "Wm": _bf16(Wm),
        "bq_sb": np.ascontiguousarray(_f32(bq).reshape(NDT, P).T),
        "bk_sb": np.ascontiguousarray(_f32(bk).reshape(NDT, P).T),
        "bv_rep": np.ascontiguousarray(np.broadcast_to(_f32(bv), (P, D))),
        "bm_rep": np.ascontiguousarray(np.broadcast_to(_f32(bm), (P, D))),
        "WgX_sb": _bf16(WgX), "WgY_sb": _bf16(WgY),
        "Wg2c": np.ascontiguousarray(
            np.broadcast_to(Wg2_f[:, :, None], (P, 2, P)).astype(NP_BF16)
        ),
        "bgX_sb": np.ascontiguousarray(_f32(bgX)[:, None]),
        "bgY_sb": np.ascontiguousarray(_f32(bgY)[:, None]),
        "bg2h": np.ascontiguousarray(
            np.broadcast_to(0.5 * _f32(bg2)[None, :], (P, 2))
        ),
    }
    def _xt(x, npdt):
        # [S, D] f32 -> x^T as [P, NDT, S] (d = i*P + p)
        xt = np.asarray(x, dtype=np.float32).T.astype(npdt)  # [D, S]
        return np.ascontiguousarray(
            xt.reshape(NDT, P, S).transpose(1, 0, 2)
        )

    in_maps = []
    for b in range(nb):
        m = dict(shared)
        m["qT"] = _xt(q[b], NP_F8)
        m["kT"] = _xt(k[b], NP_F8)
        m["vT"] = _xt(v[b], NP_BF16)
        mb = np.asarray(mask[b], dtype=np.bool_).reshape(S)
        m["maskb"] = np.ascontiguousarray(
            (mb.reshape(KJ, P).T.astype(np.float32)) * NEG
        )
        in_maps.append(m)
    return in_maps


def kernel(v, k, q, mask, Wv, bv, Wk, bk, Wq, bq, Wm, bm,
           WgX, bgX, WgY, bgY, Wg2, bg2):
    from concourse.bass_utils import run_bass_kernel_spmd

    nc = _get_nc()
    in_maps = make_in_maps(v, k, q, mask, Wv, bv, Wk, bk, Wq, bq, Wm, bm,
                           WgX, bgX, WgY, bgY, Wg2, bg2)
    res = run_bass_kernel_spmd(nc, in_maps, list(range(len(in_maps))))
    return np.stack(
        [res.results[b]["out"] for b in range(len(in_maps))]
    ).astype(np.float32)
